# revision 1
# baseline (speedup 1.0000x reference)
"""GQA attention prefill kernel for Trainium2 (Bass/Tile), 8-way tensor
parallel over heads.

Problem (hardcoded): B=1, S=2048, HID=4096, NH=32, KVH=8, D=128, causal
prefill with per-head RMSNorm on q/k and RoPE, positions = arange(S).

Sharding: core c owns kv-head c and q-heads 4c..4c+3. wq/wo sharded on the
head dim, wk/wv on the kv-head dim; x, rope tables replicated. Each core
computes its 4 heads' contribution through wo; the host sums the 8 partial
outputs.

Host-side prep (part of sharding): x and the weight shards are fed
pre-transposed (contraction dim major) so the device never transposes
activations or weights; the q/k norm weights and the rotate-half sign are
folded into transposed rope tables.

All big matmuls run in float32r (full PE rate at free dim >= 256). The
walrus verifier requires f32r matmul operands to be produced as f32r, so
every tile feeding a matmul is written with dtype float32r by its producer.
Cross-partition reductions (rms-norm sums, softmax denominators) use a
ones[128,128] lhsT, which yields the column sums already broadcast across
all 128 partitions - no degenerate M=1/N=1 matmuls and no separate
broadcast step.
"""

import numpy as np

import concourse.bass as bass
import concourse.mybir as mybir
import concourse.tile as tile
from concourse import bacc
from concourse.masks import make_identity

P = 128
S = 2048
HID = 4096
D = 128
G = 4            # q heads per core
NHT = HID // P   # 32 h-tiles (contraction)
SC = 512         # seq chunk
NSC = S // SC    # 4
NKT = S // P     # 16 k-tiles
EPS = 1e-6
N_CORES = 8

F32 = mybir.dt.float32
F32R = mybir.dt.float32r


def build_program():
    nc = bacc.Bacc("TRN2", target_bir_lowering=False, debug=False)

    xT = nc.dram_tensor("xT", [HID, S], F32R, kind="ExternalInput").ap()
    wqT = nc.dram_tensor("wqT", [HID, G * P], F32R, kind="ExternalInput").ap()
    wkT = nc.dram_tensor("wkT", [HID, P], F32R, kind="ExternalInput").ap()
    wvT = nc.dram_tensor("wvT", [HID, P], F32R, kind="ExternalInput").ap()
    woT = nc.dram_tensor("woT", [G * P, HID], F32R, kind="ExternalInput").ap()
    cosq = nc.dram_tensor("cosq", [D, S], F32, kind="ExternalInput").ap()
    sinq = nc.dram_tensor("sinq", [D, S], F32, kind="ExternalInput").ap()
    cosk = nc.dram_tensor("cosk", [D, S], F32, kind="ExternalInput").ap()
    sink = nc.dram_tensor("sink", [D, S], F32, kind="ExternalInput").ap()
    y = nc.dram_tensor("y", [S, HID], F32, kind="ExternalOutput").ap()

    Sqrt = mybir.ActivationFunctionType.Sqrt
    Exp = mybir.ActivationFunctionType.Exp

    with tile.TileContext(nc) as tc:
        with (
            tc.tile_pool(name="const", bufs=1) as const,
            tc.tile_pool(name="tabs", bufs=1) as tabs,
            tc.tile_pool(name="xw", bufs=3) as xw,
            tc.tile_pool(name="scr", bufs=2) as scr,
            tc.tile_pool(name="qrp", bufs=5) as qrp,
            tc.tile_pool(name="otp", bufs=5) as otp,
            tc.tile_pool(name="ptp", bufs=4) as ptp,
            tc.tile_pool(name="yp", bufs=3) as yp,
            tc.tile_pool(name="ps", bufs=8, space="PSUM") as ps,
        ):
            identity = const.tile([P, P], F32)
            make_identity(nc, identity)
            # gpsimd memset can't write f32r -> build constants in an f32
            # scratch and DVE-copy (rounding convert) into the f32r tiles.
            f32tmp = const.tile([P, SC], F32)
            # ones_full[k, m] == 1: matmul(out, ones_full, rhs) gives column
            # sums of rhs broadcast across all 128 output partitions.
            ones_full = const.tile([P, P], F32R)
            nc.gpsimd.memset(f32tmp, 1.0)
            nc.vector.tensor_copy(ones_full, f32tmp[:, 0:P])
            # causal masks for the 4 diagonal k-tiles of a q chunk:
            # keep (1.0) where q_local >= 128*j + k_local
            masks = []
            for j in range(4):
                mk = const.tile([P, SC], F32R, name=f"mask{j}")
                nc.gpsimd.memset(f32tmp, 1.0)
                nc.gpsimd.affine_select(
                    f32tmp, f32tmp, pattern=[[1, SC]],
                    compare_op=mybir.AluOpType.is_ge,
                    fill=0.0, base=-P * j, channel_multiplier=-1,
                )
                nc.vector.tensor_copy(mk, f32tmp)
                masks.append(mk)

            bias_keps = const.tile([P, 1], F32)
            nc.gpsimd.memset(bias_keps, float(P) * EPS)
            bias_qeps = const.tile([P, 1], F32)
            nc.gpsimd.memset(bias_qeps, EPS)

            KR = const.tile([P, S], F32R)       # roped+scaled K, [d, s]
            Vs = const.tile([P, NKT, P], F32R)  # V, [s-in-tile, k-tile, d]

            woT_sb = const.tile([P, G, HID], F32R)
            for mt in range(G):
                nc.sync.dma_start(woT_sb[:, mt, :], woT[mt * P:(mt + 1) * P, :])

            for sc in range(NSC):
                q0 = sc * SC
                cq = tabs.tile([P, SC], F32, tag="cosq")
                nc.sync.dma_start(cq, cosq[:, q0:q0 + SC])
                sq_ = tabs.tile([P, SC], F32, tag="sinq")
                nc.sync.dma_start(sq_, sinq[:, q0:q0 + SC])
                ck = tabs.tile([P, SC], F32, tag="cosk")
                nc.sync.dma_start(ck, cosk[:, q0:q0 + SC])
                sk = tabs.tile([P, SC], F32, tag="sink")
                nc.sync.dma_start(sk, sink[:, q0:q0 + SC])

                # ---- projections: accumulate over 32 h-tiles ----
                qps = [ps.tile([P, SC], F32, tag="ps", name=f"qps{i}")
                       for i in range(G)]
                kps = ps.tile([P, SC], F32, tag="ps")
                vps = ps.tile([P, SC], F32, tag="ps")
                for ht in range(NHT):
                    h0 = ht * P
                    xt = xw.tile([P, SC], F32R, tag="xt")
                    nc.sync.dma_start(xt, xT[h0:h0 + P, q0:q0 + SC])
                    wqt = xw.tile([P, G * P], F32R, tag="wqt")
                    nc.sync.dma_start(wqt, wqT[h0:h0 + P, :])
                    wkt = xw.tile([P, P], F32R, tag="wkt")
                    nc.sync.dma_start(wkt, wkT[h0:h0 + P, :])
                    wvt = xw.tile([P, P], F32R, tag="wvt")
                    nc.sync.dma_start(wvt, wvT[h0:h0 + P, :])
                    st = ht == 0
                    sp = ht == NHT - 1
                    for mt in range(G):
                        nc.tensor.matmul(
                            qps[mt], wqt[:, mt * P:(mt + 1) * P], xt,
                            start=st, stop=sp,
                        )
                    nc.tensor.matmul(kps, wkt, xt, start=st, stop=sp)
                    nc.tensor.matmul(vps, wvt, xt, start=st, stop=sp)

                # ---- K: rope + fold per-k norm scale into KR columns ----
                kraw = scr.tile([P, SC], F32, tag="kraw")
                nc.vector.tensor_copy(kraw, kps)
                sqk = scr.tile([P, SC], F32R, tag="sq")
                nc.vector.tensor_mul(sqk, kraw, kraw)
                ssb = ps.tile([P, SC], F32, tag="ps")
                nc.tensor.matmul(ssb, ones_full, sqk, start=True, stop=True)
                # 1/sqrt(ssq + d*eps) == rsqrt(mean+eps)/sqrt(d):
                # k-norm and softmax 1/sqrt(d) in one factor
                rkf = scr.tile([P, SC], F32, tag="rk")
                nc.scalar.activation(rkf, ssb, Sqrt, bias=bias_keps, scale=1.0)
                nc.vector.reciprocal(rkf, rkf)
                # rope: out = z*cos + rot(z)*sin_eff, rot(z) = [z2; z1]
                # (rotate-half minus sign folded into sin_eff on host)
                krot = scr.tile([P, SC], F32, tag="krot")
                nc.sync.dma_start(krot[0:64], kraw[64:128])
                nc.sync.dma_start(krot[64:128], kraw[0:64])
                t1 = scr.tile([P, SC], F32, tag="t1")
                nc.vector.tensor_mul(t1, krot, sk)
                kpre = scr.tile([P, SC], F32, tag="kpre")
                nc.vector.tensor_mul(kpre, kraw, ck)
                nc.vector.tensor_add(kpre, kpre, t1)
                nc.vector.tensor_mul(KR[:, q0:q0 + SC], kpre, rkf)

                # ---- V: transpose [d, s] -> [s, d] tiles ----
                vtmp = scr.tile([P, SC], F32, tag="vtmp")
                nc.vector.tensor_copy(vtmp, vps)
                for j in range(SC // P):
                    tp = ps.tile([P, P], F32, tag="ps")
                    nc.tensor.transpose(tp, vtmp[:, j * P:(j + 1) * P], identity)
                    nc.vector.tensor_copy(Vs[:, sc * 4 + j, :], tp)

                # ---- Q per head: rope + norm scale ----
                qrs = []
                for h in range(G):
                    qraw = scr.tile([P, SC], F32, tag="qraw")
                    nc.vector.tensor_copy(qraw, qps[h])
                    sqq = scr.tile([P, SC], F32R, tag="sq")
                    nc.vector.tensor_mul(sqq, qraw, qraw)
                    ssbq = ps.tile([P, SC], F32, tag="ps")
                    nc.tensor.matmul(ssbq, ones_full, sqq,
                                     start=True, stop=True)
                    rqf = scr.tile([P, SC], F32, tag="rk")
                    nc.scalar.activation(rqf, ssbq, Sqrt,
                                         bias=bias_qeps, scale=1.0 / P)
                    nc.vector.reciprocal(rqf, rqf)
                    qrot = scr.tile([P, SC], F32, tag="krot")
                    nc.sync.dma_start(qrot[0:64], qraw[64:128])
                    nc.sync.dma_start(qrot[64:128], qraw[0:64])
                    t1b = scr.tile([P, SC], F32, tag="t1")
                    nc.vector.tensor_mul(t1b, qrot, sq_)
                    qpre = scr.tile([P, SC], F32, tag="kpre")
                    nc.vector.tensor_mul(qpre, qraw, cq)
                    nc.vector.tensor_add(qpre, qpre, t1b)
                    qr = qrp.tile([P, SC], F32R, tag="qr")
                    nc.vector.tensor_mul(qr, qpre, rqf)
                    qrs.append(qr)

                # ---- attention for this q chunk ----
                ots = []
                for h in range(G):
                    avp = ps.tile([P, SC], F32, tag="ps")
                    dnp = ps.tile([P, SC], F32, tag="ps")
                    nkt = (sc + 1) * 4
                    for kt in range(nkt):
                        ptps = ps.tile([P, SC], F32, tag="ps")
                        nc.tensor.matmul(
                            ptps, KR[:, kt * P:(kt + 1) * P], qrs[h],
                            start=True, stop=True,
                        )
                        pt = ptp.tile([P, SC], F32R, tag="pt")
                        nc.scalar.activation(pt, ptps, Exp, bias=0.0,
                                             scale=1.0)
                        if kt >= sc * 4:
                            nc.vector.tensor_mul(pt, pt, masks[kt - sc * 4])
                        nc.tensor.matmul(dnp, ones_full, pt,
                                         start=(kt == 0), stop=(kt == nkt - 1))
                        nc.tensor.matmul(avp, Vs[:, kt, :], pt,
                                         start=(kt == 0), stop=(kt == nkt - 1))
                    rcp = scr.tile([P, SC], F32, tag="rcp")
                    nc.vector.reciprocal(rcp, dnp)
                    av_sb = scr.tile([P, SC], F32, tag="av")
                    nc.vector.tensor_copy(av_sb, avp)
                    ot = otp.tile([P, SC], F32R, tag="ot")
                    nc.vector.tensor_mul(ot, av_sb, rcp)
                    ots.append(ot)

                # ---- output projection for this q chunk ----
                for stl in range(SC // P):
                    srow = q0 + stl * P
                    for grp in range(2):
                        yps_l = [ps.tile([P, SC], F32, tag="ps", name=f"yps{j}")
                                 for j in range(4)]
                        for h in range(G):
                            lhs = ots[h][:, stl * P:(stl + 1) * P]
                            for j in range(4):
                                hc = grp * 4 + j
                                nc.tensor.matmul(
                                    yps_l[j], lhs,
                                    woT_sb[:, h, hc * SC:(hc + 1) * SC],
                                    start=(h == 0), stop=(h == G - 1),
                                )
                        for j in range(4):
                            hc = grp * 4 + j
                            ys = yp.tile([P, SC], F32, tag="ys")
                            nc.scalar.copy(ys, yps_l[j])
                            nc.sync.dma_start(
                                y[srow:srow + P, hc * SC:(hc + 1) * SC], ys)

    nc.finalize()
    return nc


def shard_inputs(x, wq, wk, wv, wo, q_norm_w, k_norm_w, cos_table, sin_table,
                 positions, **_ignored):
    """Host-side sharding: returns the list of 8 per-core input maps."""
    x = np.asarray(x, np.float32)
    pos = np.asarray(positions).astype(np.int64)
    cos_sel = np.asarray(cos_table, np.float32)[pos]   # [S, D]
    sin_sel = np.asarray(sin_table, np.float32)[pos]
    qw = np.asarray(q_norm_w, np.float32)
    kw = np.asarray(k_norm_w, np.float32)
    # fold norm weights into the transposed rope tables:
    # w * rope(q') == q'*(w*cos) + rot(q')*(w*sin)
    # also fold rotate-half's minus sign into sin rows 0..63:
    # rope(z) = z*cos + [-z2; z1]*sin = z*cos + [z2; z1]*sin_eff
    sign = np.ones((1, D), np.float32)
    sign[0, :D // 2] = -1.0
    cosq = np.ascontiguousarray((cos_sel * qw).T)      # [D, S]
    sinq = np.ascontiguousarray((sin_sel * qw * sign).T)
    cosk = np.ascontiguousarray((cos_sel * kw).T)
    sink = np.ascontiguousarray((sin_sel * kw * sign).T)
    xTf = np.ascontiguousarray(x.reshape(S, HID).T)    # [HID, S]
    wq = np.asarray(wq, np.float32)
    wk = np.asarray(wk, np.float32)
    wv = np.asarray(wv, np.float32)
    wo = np.asarray(wo, np.float32)

    in_maps = []
    for c in range(N_CORES):
        m = {
            "xT": xTf,
            "wqT": np.ascontiguousarray(wq[c * G * P:(c + 1) * G * P, :].T),
            "wkT": np.ascontiguousarray(wk[c * P:(c + 1) * P, :].T),
            "wvT": np.ascontiguousarray(wv[c * P:(c + 1) * P, :].T),
            "woT": np.ascontiguousarray(wo[:, c * G * P:(c + 1) * G * P].T),
            "cosq": cosq, "sinq": sinq, "cosk": cosk, "sink": sink,
        }
        in_maps.append(m)
    return in_maps


_NC = None


def _get_nc():
    global _NC
    if _NC is None:
        _NC = build_program()
    return _NC


def run_on_device(in_maps, trace=False):
    from concourse.bass_utils import run_bass_kernel_spmd
    nc = _get_nc()
    return run_bass_kernel_spmd(nc, in_maps, list(range(N_CORES)), trace=trace)


def kernel(**inputs):
    in_maps = shard_inputs(**inputs)
    res = run_on_device(in_maps).results
    y = np.zeros((S, HID), np.float32)
    for c in range(N_CORES):
        y += res[c]["y"]
    return y.reshape(1, S, HID)



# revision 5
# speedup vs baseline: 1.8637x; 1.8637x over previous
"""GQA attention prefill kernel for Trainium2 (Bass/Tile), 8-way tensor
parallel over heads.

Problem (hardcoded): B=1, S=2048, HID=4096, NH=32, KVH=8, D=128, causal
prefill with per-head RMSNorm on q/k and RoPE, positions = arange(S).

Sharding: core c owns kv-head c and q-heads 4c..4c+3. wq/wo sharded on the
head dim, wk/wv on the kv-head dim; x, rope tables replicated. Each core
computes its 4 heads' contribution through wo; the host sums the 8 partial
outputs (partials shipped as bf16, summed in fp32).

All matmul operands are bf16 (PE runs 1 cycle/row and FWL halves weight
loads; fp32r measured ~2.2 cycles/row on HW). PSUM accumulation stays fp32.
The q/k norm weights and the rotate-half sign are folded into transposed
rope tables host-side.

Structure (per chunk of 512 q positions), software-pipelined:
  proj(c+1) -> attention(c) with outproj(c-1) matmuls interleaved as PE
  filler while the scalar engine grinds exp.
Projections are head-serial (one PSUM bank at a time, x chunk resident in
SBUF) so only 2 proj banks are ever live; PSUM budget is
2 (proj+outproj) + 3 (scores) + 2 (softmax denom) + 1 (att out) = 8 banks.

RMS-norm cross-partition sums run on GpSimd (partition_all_reduce) instead
of PE ones-matmuls; reciprocals use the fast DVE approximation. The causal
diagonal k-tiles compute only the valid q columns (free-dim trim).
"""

import numpy as np

import concourse.bass as bass
import concourse.mybir as mybir
import concourse.tile as tile
from concourse import bacc
from concourse import bass_isa
from concourse.masks import make_identity

P = 128
S = 2048
HID = 4096
D = 128
G = 4            # q heads per core
NHT = HID // P   # 32 h-tiles (contraction)
SC = 512         # seq chunk
NSC = S // SC    # 4
NKT = S // P     # 16 k-tiles
EPS = 1e-6
N_CORES = 8

F32 = mybir.dt.float32
BF16 = mybir.dt.bfloat16


def build_program():
    nc = bacc.Bacc("TRN2", target_bir_lowering=False, debug=False)

    xT = nc.dram_tensor("xT", [HID, S], BF16, kind="ExternalInput").ap()
    wqT = nc.dram_tensor("wqT", [HID, G * P], BF16, kind="ExternalInput").ap()
    wkT = nc.dram_tensor("wkT", [HID, P], BF16, kind="ExternalInput").ap()
    wvT = nc.dram_tensor("wvT", [HID, P], BF16, kind="ExternalInput").ap()
    woT = nc.dram_tensor("woT", [G * P, HID], BF16, kind="ExternalInput").ap()
    cosq = nc.dram_tensor("cosq", [D, S], BF16, kind="ExternalInput").ap()
    sinq = nc.dram_tensor("sinq", [D, S], BF16, kind="ExternalInput").ap()
    cosk = nc.dram_tensor("cosk", [D, S], BF16, kind="ExternalInput").ap()
    sink = nc.dram_tensor("sink", [D, S], BF16, kind="ExternalInput").ap()
    y = nc.dram_tensor("y", [S, HID], BF16, kind="ExternalOutput").ap()

    Sqrt = mybir.ActivationFunctionType.Sqrt
    Exp = mybir.ActivationFunctionType.Exp

    with tile.TileContext(nc) as tc:
        with (
            tc.tile_pool(name="const", bufs=1) as const,
            tc.tile_pool(name="wres", bufs=1) as wres,
            tc.tile_pool(name="xp", bufs=2) as xp,
            tc.tile_pool(name="qrp", bufs=2) as qrp,
            tc.tile_pool(name="wog", bufs=3) as wogp,
            tc.tile_pool(name="scr", bufs=2) as scr,
            tc.tile_pool(name="ptp", bufs=3) as ptp,
            tc.tile_pool(name="otp", bufs=8) as otp,
            tc.tile_pool(name="rcpp", bufs=2) as rcpp,
            tc.tile_pool(name="ysp", bufs=3) as ysp,
            # PSUM: exactly 8 banks total.
            tc.tile_pool(name="psA", bufs=2, space="PSUM") as psA,  # proj+outproj
            tc.tile_pool(name="psB", bufs=3, space="PSUM") as psB,  # scores+transp
            tc.tile_pool(name="psC", bufs=2, space="PSUM") as psC,  # softmax denom
            tc.tile_pool(name="psD", bufs=1, space="PSUM") as psD,  # att out
        ):
            # ---- constants ----
            identity = const.tile([P, P], BF16)
            make_identity(nc, identity)
            f32tmp = const.tile([P, SC], F32)
            ones_bf = const.tile([P, P], BF16)
            nc.gpsimd.memset(f32tmp, 1.0)
            nc.vector.tensor_copy(ones_bf, f32tmp[:, 0:P])
            # causal masks for the 4 diagonal k-tiles of a q chunk:
            # keep (1.0) where q_local >= 128*j + k_local
            masks = []
            for j in range(4):
                mk = const.tile([P, SC], BF16, name=f"mask{j}")
                nc.gpsimd.memset(f32tmp, 1.0)
                nc.gpsimd.affine_select(
                    f32tmp, f32tmp, pattern=[[1, SC]],
                    compare_op=mybir.AluOpType.is_ge,
                    fill=0.0, base=-P * j, channel_multiplier=-1,
                )
                nc.vector.tensor_copy(mk, f32tmp)
                masks.append(mk)

            bias_keps = const.tile([P, 1], F32)
            nc.gpsimd.memset(bias_keps, float(P) * EPS)
            bias_qeps = const.tile([P, 1], F32)
            nc.gpsimd.memset(bias_qeps, EPS)

            # ---- resident tensors ----
            wq_sb = wres.tile([P, NHT, G * P], BF16)
            wk_sb = wres.tile([P, NHT, P], BF16)
            wv_sb = wres.tile([P, NHT, P], BF16)
            # interleave per-ht weight loads with chunk-0 x loads so the
            # first projection stream unblocks as early as possible
            x_tiles = [xp.tile([P, NHT, SC], BF16, tag="xall", name=f"x{c}")
                       for c in range(NSC)]
            for ht in range(NHT):
                h0 = ht * P
                nc.sync.dma_start(wq_sb[:, ht, :], wqT[h0:h0 + P, :])
                nc.sync.dma_start(wk_sb[:, ht, :], wkT[h0:h0 + P, :])
                nc.sync.dma_start(wv_sb[:, ht, :], wvT[h0:h0 + P, :])
                nc.sync.dma_start(x_tiles[0][:, ht, :], xT[h0:h0 + P, 0:SC])
            cq = wres.tile([P, S], BF16, name="cq")
            sq_ = wres.tile([P, S], BF16, name="sq")
            ck = wres.tile([P, S], BF16, name="ck")
            sk = wres.tile([P, S], BF16, name="sk")
            nc.sync.dma_start(cq, cosq)
            nc.sync.dma_start(sq_, sinq)
            nc.sync.dma_start(ck, cosk)
            nc.sync.dma_start(sk, sink)

            KR = wres.tile([P, S], BF16, name="KR")       # roped+scaled K [d, s]
            Vs = wres.tile([P, NKT, P], BF16, name="Vs")  # V [s-in-tile, kt, d]

            # ot tiles per (chunk, head) — kept alive until outproj(c) done
            ots = {}

            def emit_proj(c):
                """Projections + rope/norm for chunk c. Head-serial: one
                PSUM bank per stream, evac chains overlap the next stream."""
                q0 = c * SC
                xall = x_tiles[c]
                if c + 1 < NSC:
                    for ht in range(NHT):
                        nc.sync.dma_start(
                            x_tiles[c + 1][:, ht, :],
                            xT[ht * P:(ht + 1) * P, q0 + SC:q0 + 2 * SC])
                qr_t = qrp.tile([P, G, SC], BF16, tag="qr", name=f"qr{c}")

                def rope_norm(pj, cst, snt, bias, scale, out_ap):
                    """out = (raw*cos + rot(raw)*sin_eff) * rsqrt-factor."""
                    raw = scr.tile([P, SC], BF16, tag="raw")
                    nc.scalar.copy(raw, pj)
                    sqv = scr.tile([P, SC], BF16, tag="sqv")
                    nc.vector.tensor_mul(sqv, raw, raw)
                    ssq = scr.tile([P, SC], F32, tag="ssq")
                    nc.gpsimd.partition_all_reduce(
                        ssq, sqv, P, bass_isa.ReduceOp.add)
                    nc.scalar.activation(ssq, ssq, Sqrt, bias=bias, scale=scale)
                    rkf = scr.tile([P, SC], F32, tag="rkf")
                    nc.vector.reciprocal_approx_fast(rkf, ssq)
                    rot = scr.tile([P, SC], BF16, tag="rot")
                    nc.sync.dma_start(rot[0:64], raw[64:128])
                    nc.sync.dma_start(rot[64:128], raw[0:64])
                    t1 = scr.tile([P, SC], BF16, tag="t1")
                    nc.vector.tensor_mul(t1, rot, snt[:, q0:q0 + SC])
                    # in-place: raw = raw*cos (rot DMA read already consumed
                    # raw; WAR tracked by the framework)
                    nc.vector.tensor_mul(raw, raw, cst[:, q0:q0 + SC])
                    nc.vector.tensor_add(raw, raw, t1)
                    nc.vector.tensor_mul(out_ap, raw, rkf)

                # 4 q heads
                for h in range(G):
                    pj = psA.tile([P, SC], F32, tag="a", name=f"qp{c}_{h}")
                    for ht in range(NHT):
                        nc.tensor.matmul(
                            pj, wq_sb[:, ht, h * P:(h + 1) * P], xall[:, ht, :],
                            start=(ht == 0), stop=(ht == NHT - 1))
                    rope_norm(pj, cq, sq_, bias_qeps, 1.0 / P, qr_t[:, h, :])
                # k
                pj = psA.tile([P, SC], F32, tag="a", name=f"kp{c}")
                for ht in range(NHT):
                    nc.tensor.matmul(pj, wk_sb[:, ht, :], xall[:, ht, :],
                                     start=(ht == 0), stop=(ht == NHT - 1))
                rope_norm(pj, ck, sk, bias_keps, 1.0, KR[:, q0:q0 + SC])
                # v
                pj = psA.tile([P, SC], F32, tag="a", name=f"vp{c}")
                for ht in range(NHT):
                    nc.tensor.matmul(pj, wv_sb[:, ht, :], xall[:, ht, :],
                                     start=(ht == 0), stop=(ht == NHT - 1))
                vtmp = scr.tile([P, SC], BF16, tag="vtmp")
                nc.vector.tensor_copy(vtmp, pj)
                for j in range(SC // P):
                    tp = psB.tile([P, P], BF16, tag="b", name=f"tp{c}_{j}")
                    nc.tensor.transpose(tp, vtmp[:, j * P:(j + 1) * P], identity)
                    nc.vector.tensor_copy(Vs[:, c * 4 + j, :], tp)
                return qr_t

            def outproj_ops(c):
                """Generator of closures: output projection for chunk c,
                in PE-sized steps (one matmul per step). Used as filler
                between attention matmuls of chunk c+1."""
                q0 = c * SC
                for ng in range(HID // SC):
                    wog = wogp.tile([P, G, SC], BF16, tag="wog")
                    for h in range(G):
                        yield lambda ng=ng, h=h, wog=wog: nc.sync.dma_start(
                            wog[:, h, :],
                            woT[h * P:(h + 1) * P, ng * SC:(ng + 1) * SC])
                    for stl in range(SC // P):
                        yp = psA.tile([P, SC], F32, tag="a",
                                      name=f"yp{c}_{ng}_{stl}")
                        for h in range(G):
                            yield lambda yp=yp, h=h, stl=stl, wog=wog, c=c: \
                                nc.tensor.matmul(
                                    yp, ots[(c, h)][:, stl * P:(stl + 1) * P],
                                    wog[:, h, :],
                                    start=(h == 0), stop=(h == G - 1))

                        def evac(yp=yp, ng=ng, stl=stl, q0=q0):
                            ys = ysp.tile([P, SC], BF16, tag="ys")
                            nc.any.tensor_copy(ys, yp)
                            nc.sync.dma_start(
                                y[q0 + stl * P:q0 + (stl + 1) * P,
                                  ng * SC:(ng + 1) * SC], ys)
                        yield evac

            def emit_attn(c, qr_t, filler):
                """Attention for chunk c; `filler` ops (outproj of c-1)
                are interleaved to keep PE busy while ACT runs exp."""
                def take(n):
                    for _ in range(n):
                        op = next(filler, None)
                        if op is None:
                            return
                        op()

                nkt = (c + 1) * 4
                for h in range(G):
                    avp = psD.tile([P, SC], F32, tag="d", name=f"av{c}_{h}")
                    dnp = psC.tile([P, SC], F32, tag="c", name=f"dn{c}_{h}")
                    for kt in range(nkt):
                        j = kt - c * 4
                        off = P * j if j >= 0 else 0
                        ptps = psB.tile([P, SC], F32, tag="b",
                                        name=f"pt{c}_{h}_{kt}")
                        nc.tensor.matmul(
                            ptps[:, off:], KR[:, kt * P:(kt + 1) * P],
                            qr_t[:, h, off:], start=True, stop=True)
                        pt = ptp.tile([P, SC], BF16, tag="pt")
                        nc.scalar.activation(pt[:, off:], ptps[:, off:], Exp,
                                             bias=0.0, scale=1.0)
                        if j >= 0:
                            nc.vector.tensor_mul(pt[:, off:], pt[:, off:],
                                                 masks[j][:, off:])
                        nc.tensor.matmul(dnp[:, off:], ones_bf, pt[:, off:],
                                         start=(kt == 0), stop=(kt == nkt - 1))
                        nc.tensor.matmul(avp[:, off:], Vs[:, kt, :],
                                         pt[:, off:],
                                         start=(kt == 0), stop=(kt == nkt - 1))
                        take(2)
                    rcp = rcpp.tile([P, SC], F32, tag="rcp")
                    nc.vector.reciprocal_approx_fast(rcp, dnp)
                    ot = otp.tile([P, SC], BF16, tag="ot", name=f"ot{c}_{h}")
                    nc.vector.tensor_mul(ot, avp, rcp)
                    ots[(c, h)] = ot
                    take(8)

            # ---- pipelined main loop ----
            qr_next = emit_proj(0)
            empty = iter(())
            for c in range(NSC):
                qr_cur = qr_next
                if c + 1 < NSC:
                    qr_next = emit_proj(c + 1)
                filler = outproj_ops(c - 1) if c >= 1 else empty
                emit_attn(c, qr_cur, filler)
                for op in filler:  # flush leftovers
                    op()
            for op in outproj_ops(NSC - 1):
                op()

    nc.finalize()
    return nc


def shard_inputs(x, wq, wk, wv, wo, q_norm_w, k_norm_w, cos_table, sin_table,
                 positions, **_ignored):
    """Host-side sharding: returns the list of 8 per-core input maps."""
    import ml_dtypes
    bf16 = ml_dtypes.bfloat16

    x = np.asarray(x, np.float32)
    pos = np.asarray(positions).astype(np.int64)
    cos_sel = np.asarray(cos_table, np.float32)[pos]   # [S, D]
    sin_sel = np.asarray(sin_table, np.float32)[pos]
    qw = np.asarray(q_norm_w, np.float32)
    kw = np.asarray(k_norm_w, np.float32)
    # fold norm weights into the transposed rope tables:
    # w * rope(q') == q'*(w*cos) + rot(q')*(w*sin)
    # also fold rotate-half's minus sign into sin rows 0..63:
    # rope(z) = z*cos + [-z2; z1]*sin = z*cos + [z2; z1]*sin_eff
    sign = np.ones((1, D), np.float32)
    sign[0, :D // 2] = -1.0
    cosq = np.ascontiguousarray((cos_sel * qw).T).astype(bf16)      # [D, S]
    sinq = np.ascontiguousarray((sin_sel * qw * sign).T).astype(bf16)
    cosk = np.ascontiguousarray((cos_sel * kw).T).astype(bf16)
    sink = np.ascontiguousarray((sin_sel * kw * sign).T).astype(bf16)
    xTf = np.ascontiguousarray(x.reshape(S, HID).T).astype(bf16)    # [HID, S]
    wq = np.asarray(wq, np.float32)
    wk = np.asarray(wk, np.float32)
    wv = np.asarray(wv, np.float32)
    wo = np.asarray(wo, np.float32)

    in_maps = []
    for c in range(N_CORES):
        m = {
            "xT": xTf,
            "wqT": np.ascontiguousarray(
                wq[c * G * P:(c + 1) * G * P, :].T).astype(bf16),
            "wkT": np.ascontiguousarray(
                wk[c * P:(c + 1) * P, :].T).astype(bf16),
            "wvT": np.ascontiguousarray(
                wv[c * P:(c + 1) * P, :].T).astype(bf16),
            "woT": np.ascontiguousarray(
                wo[:, c * G * P:(c + 1) * G * P].T).astype(bf16),
            "cosq": cosq, "sinq": sinq, "cosk": cosk, "sink": sink,
        }
        in_maps.append(m)
    return in_maps


_NC = None


def _get_nc():
    global _NC
    if _NC is None:
        _NC = build_program()
    return _NC


def run_on_device(in_maps, trace=False):
    from concourse.bass_utils import run_bass_kernel_spmd
    nc = _get_nc()
    return run_bass_kernel_spmd(nc, in_maps, list(range(N_CORES)), trace=trace)


def kernel(**inputs):
    in_maps = shard_inputs(**inputs)
    res = run_on_device(in_maps).results
    y = np.zeros((S, HID), np.float32)
    for c in range(N_CORES):
        y += np.asarray(res[c]["y"], np.float32)
    return y.reshape(1, S, HID)


# revision 7
# speedup vs baseline: 2.2124x; 1.1871x over previous
"""GQA attention prefill kernel for Trainium2 (Bass/Tile), 8-way tensor
parallel over heads.

Problem (hardcoded): B=1, S=2048, HID=4096, NH=32, KVH=8, D=128, causal
prefill with per-head RMSNorm on q/k and RoPE, positions = arange(S).

Sharding: core c owns kv-head c and q-heads 4c..4c+3. wq/wo sharded on the
head dim, wk/wv on the kv-head dim; x, rope tables replicated. Each core
computes its 4 heads' contribution through wo; the host sums the 8 partial
outputs (partials shipped as bf16, summed in fp32).

All matmul operands are bf16 (PE runs 1 cycle/row and FWL halves weight
loads; fp32r measured ~2.2 cycles/row on HW). PSUM accumulation stays fp32.
The rotate-half sign is folded into the sin table host-side; the q/k norm
weights are applied on-device as a per-partition scalar in the fused
(pre * w) * rsqrt multiply.

Weights/activations are shipped in [partition, tile, free] 3-D layouts so
every SBUF load is one strided DMA descriptor (the Sync sequencer pays
~600 ns per dma_start; v1 of this kernel lost ~250 us to descriptor issue).
Big transfers are split across a few descriptors so multiple DMA engines
run in parallel (one queue sustains only ~24 GB/s).

Structure (per chunk of 512 q positions), software-pipelined:
  proj(c+1) -> attention(c) with outproj(c-1) matmuls interleaved as PE
  filler while the scalar engine grinds exp.
Projections are head-serial (one PSUM bank at a time, x chunk resident in
SBUF) so only 2 proj banks are ever live; PSUM budget is
2 (proj+outproj) + 3 (scores) + 2 (softmax denom) + 1 (att out) = 8 banks.

RMS-norm cross-partition sums run on GpSimd (partition_all_reduce) instead
of PE ones-matmuls; reciprocals use the fast DVE approximation. The causal
diagonal k-tiles compute only the valid q columns (free-dim trim).
"""

import numpy as np

import concourse.bass as bass
import concourse.mybir as mybir
import concourse.tile as tile
from concourse import bacc
from concourse import bass_isa
from concourse.masks import make_identity

P = 128
S = 2048
HID = 4096
D = 128
G = 4            # q heads per core
NHT = HID // P   # 32 h-tiles (contraction)
SC = 512         # seq chunk
NSC = S // SC    # 4
NKT = S // P     # 16 k-tiles
EPS = 1e-6
N_CORES = 8

F32 = mybir.dt.float32
BF16 = mybir.dt.bfloat16
MULT = mybir.AluOpType.mult


def build_program():
    nc = bacc.Bacc("TRN2", target_bir_lowering=False, debug=False)

    xT = nc.dram_tensor("xT", [P, NHT, S], BF16, kind="ExternalInput").ap()
    wqT = nc.dram_tensor("wqT", [P, NHT, G * P], BF16,
                         kind="ExternalInput").ap()
    wkT = nc.dram_tensor("wkT", [P, NHT, P], BF16, kind="ExternalInput").ap()
    wvT = nc.dram_tensor("wvT", [P, NHT, P], BF16, kind="ExternalInput").ap()
    woT = nc.dram_tensor("woT", [P, G, HID], BF16, kind="ExternalInput").ap()
    cost = nc.dram_tensor("cost", [D, S], BF16, kind="ExternalInput").ap()
    sint = nc.dram_tensor("sint", [D, S], BF16, kind="ExternalInput").ap()
    qnw = nc.dram_tensor("qnw", [D, 1], F32, kind="ExternalInput").ap()
    knw = nc.dram_tensor("knw", [D, 1], F32, kind="ExternalInput").ap()
    y = nc.dram_tensor("y", [S, HID], BF16, kind="ExternalOutput").ap()

    Sqrt = mybir.ActivationFunctionType.Sqrt
    Exp = mybir.ActivationFunctionType.Exp

    with tile.TileContext(nc) as tc:
        with (
            tc.tile_pool(name="const", bufs=1) as const,
            tc.tile_pool(name="wres", bufs=1) as wres,
            tc.tile_pool(name="xp", bufs=2) as xp,
            tc.tile_pool(name="qrp", bufs=2) as qrp,
            tc.tile_pool(name="wog", bufs=3) as wogp,
            tc.tile_pool(name="scr", bufs=2) as scr,
            tc.tile_pool(name="ptp", bufs=3) as ptp,
            tc.tile_pool(name="otp", bufs=8) as otp,
            tc.tile_pool(name="rcpp", bufs=2) as rcpp,
            tc.tile_pool(name="ysp", bufs=2) as ysp,
            # PSUM: exactly 8 banks total.
            tc.tile_pool(name="psA", bufs=2, space="PSUM") as psA,  # proj+outproj
            tc.tile_pool(name="psB", bufs=3, space="PSUM") as psB,  # scores+transp
            tc.tile_pool(name="psC", bufs=2, space="PSUM") as psC,  # softmax denom
            tc.tile_pool(name="psD", bufs=1, space="PSUM") as psD,  # att out
        ):
            # ---- resident tensors (batched loads, interleaved so the
            # first projection stream unblocks early) ----
            wq_sb = wres.tile([P, NHT, G * P], BF16)
            wk_sb = wres.tile([P, NHT, P], BF16)
            wv_sb = wres.tile([P, NHT, P], BF16)
            x_tiles = [xp.tile([P, NHT, SC], BF16, tag="xall", name=f"x{c}")
                       for c in range(NSC)]
            for i in range(8):
                h4 = slice(i * 4, (i + 1) * 4)
                nc.sync.dma_start(x_tiles[0][:, h4, :], xT[:, h4, 0:SC])
                nc.sync.dma_start(wq_sb[:, h4, :], wqT[:, h4, :])
            nc.sync.dma_start(wk_sb[:, 0:16, :], wkT[:, 0:16, :])
            nc.sync.dma_start(wk_sb[:, 16:32, :], wkT[:, 16:32, :])
            nc.sync.dma_start(wv_sb[:, 0:16, :], wvT[:, 0:16, :])
            nc.sync.dma_start(wv_sb[:, 16:32, :], wvT[:, 16:32, :])
            cs = wres.tile([P, S], BF16, name="cs")
            sn = wres.tile([P, S], BF16, name="sn")
            nc.sync.dma_start(cs[:, 0:S // 2], cost[:, 0:S // 2])
            nc.sync.dma_start(cs[:, S // 2:S], cost[:, S // 2:S])
            nc.sync.dma_start(sn[:, 0:S // 2], sint[:, 0:S // 2])
            nc.sync.dma_start(sn[:, S // 2:S], sint[:, S // 2:S])
            qn = const.tile([P, 1], F32, name="qn")
            kn = const.tile([P, 1], F32, name="kn")
            nc.sync.dma_start(qn, qnw)
            nc.sync.dma_start(kn, knw)

            # ---- constants ----
            identity = const.tile([P, P], BF16)
            make_identity(nc, identity)
            f32tmp = const.tile([P, SC], F32)
            ones_bf = const.tile([P, P], BF16)
            nc.gpsimd.memset(f32tmp, 1.0)
            nc.vector.tensor_copy(ones_bf, f32tmp[:, 0:P])
            # causal masks for the 4 diagonal k-tiles of a q chunk:
            # keep (1.0) where q_local >= 128*j + k_local
            masks = []
            for j in range(4):
                mk = const.tile([P, SC], BF16, name=f"mask{j}")
                nc.gpsimd.memset(f32tmp, 1.0)
                nc.gpsimd.affine_select(
                    f32tmp, f32tmp, pattern=[[1, SC]],
                    compare_op=mybir.AluOpType.is_ge,
                    fill=0.0, base=-P * j, channel_multiplier=-1,
                )
                nc.vector.tensor_copy(mk, f32tmp)
                masks.append(mk)

            bias_keps = const.tile([P, 1], F32)
            nc.gpsimd.memset(bias_keps, float(P) * EPS)
            bias_qeps = const.tile([P, 1], F32)
            nc.gpsimd.memset(bias_qeps, EPS)

            KR = wres.tile([P, S], BF16, name="KR")       # roped+scaled K [d, s]
            Vs = wres.tile([P, NKT, P], BF16, name="Vs")  # V [s-in-tile, kt, d]

            # ot tiles per (chunk, head) — kept alive until outproj(c) done
            ots = {}

            def emit_proj(c):
                """Projections + rope/norm for chunk c. Head-serial: one
                PSUM bank per stream, evac chains overlap the next stream."""
                q0 = c * SC
                xall = x_tiles[c]
                if c + 1 < NSC:
                    for i in range(8):
                        h4 = slice(i * 4, (i + 1) * 4)
                        nc.sync.dma_start(x_tiles[c + 1][:, h4, :],
                                          xT[:, h4, q0 + SC:q0 + 2 * SC])
                qr_t = qrp.tile([P, G, SC], BF16, tag="qr", name=f"qr{c}")

                def rope_norm(pj, nw, bias, scale, out_ap):
                    """out = ((raw*cos + rot(raw)*sin_eff) * norm_w) * rsqrt."""
                    raw = scr.tile([P, SC], BF16, tag="raw")
                    nc.scalar.copy(raw, pj)
                    sqv = scr.tile([P, SC], BF16, tag="sqv")
                    nc.vector.tensor_mul(sqv, raw, raw)
                    ssq = scr.tile([P, SC], F32, tag="ssq")
                    nc.gpsimd.partition_all_reduce(
                        ssq, sqv, P, bass_isa.ReduceOp.add)
                    nc.scalar.activation(ssq, ssq, Sqrt, bias=bias, scale=scale)
                    rkf = scr.tile([P, SC], F32, tag="rkf")
                    nc.vector.reciprocal_approx_fast(rkf, ssq)
                    rot = scr.tile([P, SC], BF16, tag="rot")
                    nc.sync.dma_start(rot[0:64], raw[64:128])
                    nc.sync.dma_start(rot[64:128], raw[0:64])
                    t1 = scr.tile([P, SC], BF16, tag="t1")
                    nc.vector.tensor_mul(t1, rot, sn[:, q0:q0 + SC])
                    # in-place: raw = raw*cos (rot DMA read already consumed
                    # raw; WAR tracked by the framework)
                    nc.vector.tensor_mul(raw, raw, cs[:, q0:q0 + SC])
                    nc.vector.tensor_add(raw, raw, t1)
                    # out = (raw * norm_w[P,1]) * rsqrt_factor, one DVE op
                    nc.vector.scalar_tensor_tensor(
                        out_ap, raw, nw, rkf, MULT, MULT)

                # 4 q heads
                for h in range(G):
                    pj = psA.tile([P, SC], F32, tag="a", name=f"qp{c}_{h}")
                    for ht in range(NHT):
                        nc.tensor.matmul(
                            pj, wq_sb[:, ht, h * P:(h + 1) * P], xall[:, ht, :],
                            start=(ht == 0), stop=(ht == NHT - 1))
                    rope_norm(pj, qn, bias_qeps, 1.0 / P, qr_t[:, h, :])
                # k
                pj = psA.tile([P, SC], F32, tag="a", name=f"kp{c}")
                for ht in range(NHT):
                    nc.tensor.matmul(pj, wk_sb[:, ht, :], xall[:, ht, :],
                                     start=(ht == 0), stop=(ht == NHT - 1))
                rope_norm(pj, kn, bias_keps, 1.0, KR[:, q0:q0 + SC])
                # v
                pj = psA.tile([P, SC], F32, tag="a", name=f"vp{c}")
                for ht in range(NHT):
                    nc.tensor.matmul(pj, wv_sb[:, ht, :], xall[:, ht, :],
                                     start=(ht == 0), stop=(ht == NHT - 1))
                vtmp = scr.tile([P, SC], BF16, tag="raw")
                nc.vector.tensor_copy(vtmp, pj)
                for j in range(SC // P):
                    tp = psB.tile([P, P], BF16, tag="b", name=f"tp{c}_{j}")
                    nc.tensor.transpose(tp, vtmp[:, j * P:(j + 1) * P], identity)
                    nc.vector.tensor_copy(Vs[:, c * 4 + j, :], tp)
                return qr_t

            def outproj_ops(c):
                """Generator of closures: output projection for chunk c,
                in PE-sized steps (one matmul per step). Used as filler
                between attention matmuls of chunk c+1."""
                q0 = c * SC
                for ng in range(HID // SC):
                    wog = wogp.tile([P, G, SC], BF16, tag="wog")
                    yield lambda ng=ng, wog=wog: nc.sync.dma_start(
                        wog, woT[:, :, ng * SC:(ng + 1) * SC])
                    ys = ysp.tile([P, SC // P, SC], BF16, tag="ys")
                    for stl in range(SC // P):
                        yp = psA.tile([P, SC], F32, tag="a",
                                      name=f"yp{c}_{ng}_{stl}")
                        for h in range(G):
                            yield lambda yp=yp, h=h, stl=stl, wog=wog, c=c: \
                                nc.tensor.matmul(
                                    yp, ots[(c, h)][:, stl * P:(stl + 1) * P],
                                    wog[:, h, :],
                                    start=(h == 0), stop=(h == G - 1))
                        yield lambda ys=ys, yp=yp, stl=stl: \
                            nc.any.tensor_copy(ys[:, stl, :], yp)
                    # one strided store for the whole [SC, SC] block:
                    # y[q0+stl*128+p, ng*512+n] = ys[p, stl, n]
                    yield lambda ys=ys, ng=ng, q0=q0: nc.sync.dma_start(
                        y[q0:q0 + SC, ng * SC:(ng + 1) * SC].rearrange(
                            "(stl p) n -> p stl n", stl=SC // P, p=P), ys)

            def emit_attn(c, qr_t, filler):
                """Attention for chunk c; `filler` ops (outproj of c-1)
                are interleaved to keep PE busy while ACT runs exp."""
                def take(n):
                    for _ in range(n):
                        op = next(filler, None)
                        if op is None:
                            return
                        op()

                nkt = (c + 1) * 4
                for h in range(G):
                    avp = psD.tile([P, SC], F32, tag="d", name=f"av{c}_{h}")
                    dnp = psC.tile([P, SC], F32, tag="c", name=f"dn{c}_{h}")
                    for kt in range(nkt):
                        j = kt - c * 4
                        off = P * j if j >= 0 else 0
                        ptps = psB.tile([P, SC], F32, tag="b",
                                        name=f"pt{c}_{h}_{kt}")
                        nc.tensor.matmul(
                            ptps[:, off:], KR[:, kt * P:(kt + 1) * P],
                            qr_t[:, h, off:], start=True, stop=True)
                        pt = ptp.tile([P, SC], BF16, tag="pt")
                        nc.scalar.activation(pt[:, off:], ptps[:, off:], Exp,
                                             bias=0.0, scale=1.0)
                        if j >= 0:
                            nc.vector.tensor_mul(pt[:, off:], pt[:, off:],
                                                 masks[j][:, off:])
                        nc.tensor.matmul(dnp[:, off:], ones_bf, pt[:, off:],
                                         start=(kt == 0), stop=(kt == nkt - 1))
                        nc.tensor.matmul(avp[:, off:], Vs[:, kt, :],
                                         pt[:, off:],
                                         start=(kt == 0), stop=(kt == nkt - 1))
                        take(2)
                    rcp = rcpp.tile([P, SC], F32, tag="rcp")
                    nc.vector.reciprocal_approx_fast(rcp, dnp)
                    ot = otp.tile([P, SC], BF16, tag="ot", name=f"ot{c}_{h}")
                    nc.vector.tensor_mul(ot, avp, rcp)
                    ots[(c, h)] = ot
                    take(7)

            # ---- pipelined main loop ----
            qr_next = emit_proj(0)
            empty = iter(())
            for c in range(NSC):
                qr_cur = qr_next
                if c + 1 < NSC:
                    qr_next = emit_proj(c + 1)
                filler = outproj_ops(c - 1) if c >= 1 else empty
                emit_attn(c, qr_cur, filler)
                for op in filler:  # flush leftovers
                    op()
            for op in outproj_ops(NSC - 1):
                op()

    nc.finalize()
    return nc


def shard_inputs(x, wq, wk, wv, wo, q_norm_w, k_norm_w, cos_table, sin_table,
                 positions, **_ignored):
    """Host-side sharding: returns the list of 8 per-core input maps."""
    import ml_dtypes
    bf16 = ml_dtypes.bfloat16

    x = np.asarray(x, np.float32)
    pos = np.asarray(positions).astype(np.int64)
    cos_sel = np.asarray(cos_table, np.float32)[pos]   # [S, D]
    sin_sel = np.asarray(sin_table, np.float32)[pos]
    qw = np.ascontiguousarray(
        np.asarray(q_norm_w, np.float32).reshape(D, 1))
    kw = np.ascontiguousarray(
        np.asarray(k_norm_w, np.float32).reshape(D, 1))
    # fold rotate-half's minus sign into sin rows 0..63:
    # rope(z) = z*cos + [-z2; z1]*sin = z*cos + [z2; z1]*sin_eff
    sign = np.ones((1, D), np.float32)
    sign[0, :D // 2] = -1.0
    cost = np.ascontiguousarray(cos_sel.T).astype(bf16)            # [D, S]
    sint = np.ascontiguousarray((sin_sel * sign).T).astype(bf16)
    # x as [p, ht, s]: x[s, ht*128+p]
    xT3 = np.ascontiguousarray(
        x.reshape(S, NHT, P).transpose(2, 1, 0)).astype(bf16)
    wq = np.asarray(wq, np.float32)
    wk = np.asarray(wk, np.float32)
    wv = np.asarray(wv, np.float32)
    wo = np.asarray(wo, np.float32)

    in_maps = []
    for c in range(N_CORES):
        # weight shards, [p, ht, m] with p the contraction partition
        wq_s = wq[c * G * P:(c + 1) * G * P, :].T     # [HID, 512]
        wk_s = wk[c * P:(c + 1) * P, :].T             # [HID, 128]
        wv_s = wv[c * P:(c + 1) * P, :].T
        wo_s = wo[:, c * G * P:(c + 1) * G * P].T     # [512, HID]
        m = {
            "xT": xT3,
            "wqT": np.ascontiguousarray(
                wq_s.reshape(NHT, P, G * P).transpose(1, 0, 2)).astype(bf16),
            "wkT": np.ascontiguousarray(
                wk_s.reshape(NHT, P, P).transpose(1, 0, 2)).astype(bf16),
            "wvT": np.ascontiguousarray(
                wv_s.reshape(NHT, P, P).transpose(1, 0, 2)).astype(bf16),
            "woT": np.ascontiguousarray(
                wo_s.reshape(G, P, HID).transpose(1, 0, 2)).astype(bf16),
            "cost": cost, "sint": sint, "qnw": qw, "knw": kw,
        }
        in_maps.append(m)
    return in_maps


_NC = None


def _get_nc():
    global _NC
    if _NC is None:
        _NC = build_program()
    return _NC


def run_on_device(in_maps, trace=False):
    from concourse.bass_utils import run_bass_kernel_spmd
    nc = _get_nc()
    return run_bass_kernel_spmd(nc, in_maps, list(range(N_CORES)), trace=trace)


def kernel(**inputs):
    in_maps = shard_inputs(**inputs)
    res = run_on_device(in_maps).results
    y = np.zeros((S, HID), np.float32)
    for c in range(N_CORES):
        y += np.asarray(res[c]["y"], np.float32)
    return y.reshape(1, S, HID)
